# revision 11
# baseline (speedup 1.0000x reference)
"""Binary position embedding kernel for Trainium2 (8 NeuronCores, SPMD).

out[t, :] = sum_{b : bit b of x[t] set} emb[b, :]   ==   mask(x) @ emb

Strategy (data-parallel over tokens, per the sharding hint):
  - Flatten x (4, 8192) -> (32768,), shard 4096 tokens per core; the
    (tiny) emb table is replicated.  Each core computes its (4096, 1024)
    output slab; host concatenates.
  - The output is written as bf16 and upcast to f32 on the host: the
    kernel is memory-bound on output writes, and bf16 halves the 16 MiB
    of f32 traffic per core while its <=2^-9 relative rounding error is
    ~10x under the 2e-2 gate.  The measured per-core DMA write plateau
    is ~325 GB/s (HBM share; a second HWDGE queue adds only ~3%), so
    8 MiB of bf16 writes floor the kernel at ~26 us.
  - PE row tiling 2-way: token chunks A/B have their bit rows at SBUF
    partitions 0-63 / 64-127, so consecutive matmuls land in different
    PE row-groups and execute concurrently (measured 2.4x: 64 N=512
    matmuls in 14.3 us vs 34.4 flat).  emb is split hi/lo into bf16
    (hi = bf16(emb), lo = bf16(emb - hi), ~2^-16 combined error) at
    partitions 0/32 for chunk A and duplicated at 64/96 for chunk B.
  - t = (x + 0.25) * 2^-(b+1) is computed BY THE PE: a K=3 matmul per
    chunk with stationary pwcol[k, m] = 2^-(b(m)+1) and moving rows
    [bf16(x), x - bf16(x), 0.25] -- every product is a power-of-two
    scaled bf16, so the f32 PSUM result is exact.  This removes the
    Pool broadcast (whose write is base-0-only) and one DVE op.
  - DVE finishes the mask: r = (t + 2^23) - 2^23 (RNE round, tie-free
    thanks to the +0.25), bit = (t < r), written as the bf16 mask tile.
    Rows with pw=0 yield exactly-zero mask rows, so unused partitions
    are inert.
  - Per 128-token j-tile: 2 matmuls into a [128, 1024] 2-bank PSUM
    tile, one PSUM->SBUF bf16-converting copy (split DVE/ACT per
    COPY_W), one 256 KiB output DMA on the SP HWDGE queue.
"""

import sys

import numpy as np

if "/opt/trn_rl_repo" not in sys.path:
    sys.path.insert(0, "/opt/trn_rl_repo")

N_BITS = 13
D_MODEL = 1024
N_CORES = 8
TOKENS = 4 * 8192
TOK_PER_CORE = TOKENS // N_CORES  # 4096

# Chunk-A bit b at partition b (hi) / 32+b (lo); chunk-B at 64+b / 96+b.
KP = 128
LO_OFF = 32
MMT = 128  # tokens per matmul (output partition dim)
N_J = TOK_PER_CORE // MMT  # 32 j-tiles

GROUPS = [256, 256] + [512] * 7  # token groups; sum = TOK_PER_CORE
# Pool cannot access PSUM on TRN2, so the PSUM->SBUF copies are split
# between DVE and ACT; SP issues every output DMA.
COPY_W = (13, 19, 0)  # j-tile copy split (DVE, ACT, Pool)
DMA_PAT = "s"  # per-j output DMA queue: s=SP(sync) a=ACT(scalar)
PSUM_BUFS = 3  # [128,1024] 2-bank tiles (6 banks; +2 for the t tiles)
TPS_BUFS = 2
OUTP_BUFS = 6
MASKP_BUFS = 3
STAGGERED_RESET = False  # benchmark loop back-edge mode

_CACHE = {}
last_results = None  # BassKernelResults of the most recent run (for test.py)


def _copy_engines():
    """Spread COPY_W copies per engine evenly over the N_J j-tiles."""
    used = [0, 0, 0]
    out = []
    for j in range(N_J):
        deficits = [COPY_W[e] * (j + 1) / N_J - used[e] for e in range(3)]
        e = max(range(3), key=lambda i: deficits[i])
        used[e] += 1
        out.append(e)
    return out


def _build_module(loop_reps=None):
    """Build the per-core Bass module.

    loop_reps: if set, wrap the whole pipeline in a tc.For_i repetition
    loop (benchmark-only; ~2us back-edge per iteration).
    """
    import concourse.bacc as bacc
    import concourse.mybir as mybir
    import concourse.tile as tile
    from contextlib import ExitStack

    f32 = mybir.dt.float32
    bf16 = mybir.dt.bfloat16

    nc = bacc.Bacc("TRN2", target_bir_lowering=False)

    assert sum(GROUPS) == TOK_PER_CORE
    xhl_d = nc.dram_tensor("xhl", [3, TOK_PER_CORE], bf16, kind="ExternalInput")
    pwcol_d = nc.dram_tensor("pwcol", [3, KP], bf16, kind="ExternalInput")
    embhl_d = nc.dram_tensor("embhl", [KP, D_MODEL], bf16, kind="ExternalInput")
    out_d = nc.dram_tensor("out", [TOK_PER_CORE, D_MODEL], bf16, kind="ExternalOutput")

    # DRAM view [p, j, d]: token index = j*MMT + p  (j counts MMT tiles)
    out_pjd = out_d.rearrange("(j p) d -> p j d", p=MMT)

    copy_eng = _copy_engines()
    max_cols = max(GROUPS) // 2

    with ExitStack() as ctx:
        tc = ctx.enter_context(tile.TileContext(nc))
        if loop_reps is not None:
            ctx.enter_context(
                tc.For_i(0, loop_reps, 1, staggered_reset=STAGGERED_RESET)
            )
        const = ctx.enter_context(tc.tile_pool(name="const", bufs=1))
        maskp = ctx.enter_context(tc.tile_pool(name="maskp", bufs=MASKP_BUFS))
        tpsp = ctx.enter_context(tc.tile_pool(name="tps", bufs=TPS_BUFS, space="PSUM"))
        psum = ctx.enter_context(tc.tile_pool(name="psum", bufs=PSUM_BUFS, space="PSUM"))
        outp = ctx.enter_context(tc.tile_pool(name="outp", bufs=OUTP_BUFS))

        # --- constants ---  (xhl gates the first t-matmul: it goes first)
        xhl_sb = const.tile([3, TOK_PER_CORE], bf16)
        nc.sync.dma_start(xhl_sb[:], xhl_d[:])
        pwcol_t = const.tile([3, KP], bf16)
        nc.sync.dma_start(pwcol_t[:], pwcol_d[:])
        emb_hl = const.tile([KP, D_MODEL], bf16)
        nc.scalar.dma_start(emb_hl[:], embhl_d[:])

        # ACT warm-up: force the activation-function table load (~1.3us)
        # off the first real copy's critical path
        warm_act = const.tile([3, 8], bf16)
        nc.scalar.copy(warm_act[:], pwcol_t[:, 0:8])

        # --- main loop ---
        tok0 = 0
        for gtok in GROUPS:
            cols = gtok // 2  # tokens per chunk == mask columns

            # t[m, n] = pwcol[m] * (x_n + 0.25), exact, PE-computed
            tps = tpsp.tile([KP, max_cols], f32, tag="tps")
            for c in range(2):
                nc.tensor.matmul(
                    tps[64 * c : 64 * c + 64, 0:cols],
                    pwcol_t[:, 64 * c : 64 * c + 64],
                    xhl_sb[:, tok0 + c * cols : tok0 + (c + 1) * cols],
                    start=True,
                    stop=True,
                )
            r = maskp.tile([KP, max_cols], f32, tag="r")
            nc.vector.tensor_scalar(
                out=r[:, 0:cols],
                in0=tps[:, 0:cols],
                scalar1=float(2**23),
                scalar2=float(2**23),
                op0=mybir.AluOpType.add,
                op1=mybir.AluOpType.subtract,
            )
            mask = maskp.tile([KP, max_cols], bf16, tag="mask")
            nc.vector.tensor_tensor(
                out=mask[:, 0:cols],
                in0=tps[:, 0:cols],
                in1=r[:, 0:cols],
                op=mybir.AluOpType.is_lt,
            )

            for jj in range(cols // MMT):
                pss = [
                    psum.tile([MMT, D_MODEL], f32, tag="ps", name=f"ps{c}")
                    for c in range(2)
                ]
                for h in range(2):
                    for c in range(2):
                        nc.tensor.matmul(
                            pss[c][:, h * 512 : (h + 1) * 512],
                            mask[64 * c : 64 * c + 64, jj * MMT : (jj + 1) * MMT],
                            emb_hl[64 * c : 64 * c + 64, h * 512 : (h + 1) * 512],
                            start=True,
                            stop=True,
                        )
                for c in range(2):
                    jg = (tok0 + c * cols) // MMT + jj
                    ob = outp.tile([MMT, D_MODEL], bf16, tag="ob")
                    ce = copy_eng[jg]
                    if ce == 0:
                        nc.vector.tensor_copy(ob[:], pss[c][:])
                    else:
                        nc.scalar.copy(ob[:], pss[c][:])
                    dq = DMA_PAT[jg % len(DMA_PAT)]
                    deng = nc.sync if dq == "s" else nc.scalar
                    deng.dma_start(out_pjd[:, jg], ob[:])
            tok0 += gtok

    nc.compile()
    return nc


def _get_module():
    if "nc" not in _CACHE:
        _CACHE["nc"] = _build_module()
    return _CACHE["nc"]


def _make_consts(emb):
    """Host-precomputed constant tables: the K=3 stationary pw columns and
    the hi/lo bf16 split of emb at partitions 0/32 (chunk A) and 64/96
    (chunk B)."""
    import ml_dtypes

    pwvec = np.zeros(KP, dtype=np.float32)
    bits = np.arange(N_BITS, dtype=np.float64)
    for off in (0, LO_OFF, 64, 64 + LO_OFF):
        pwvec[off : off + N_BITS] = 2.0 ** -(bits + 1.0)
    pwcol = np.broadcast_to(pwvec, (3, KP)).astype(ml_dtypes.bfloat16)

    emb = np.asarray(emb, dtype=np.float32)
    hi = emb.astype(ml_dtypes.bfloat16)
    lo = (emb - hi.astype(np.float32)).astype(ml_dtypes.bfloat16)
    embhl = np.zeros((KP, D_MODEL), dtype=ml_dtypes.bfloat16)
    for off in (0, 64):
        embhl[off : off + N_BITS] = hi
        embhl[off + LO_OFF : off + LO_OFF + N_BITS] = lo
    return pwcol, embhl


def _make_in_maps(x_f32, emb):
    """Per-core input dicts: exact bf16 hi/lo split of the x shard (plus a
    0.25 row) and the const tables."""
    import ml_dtypes

    pwcol, embhl = _make_consts(emb)
    in_maps = []
    for c in range(N_CORES):
        shard = x_f32[c * TOK_PER_CORE : (c + 1) * TOK_PER_CORE]
        xhi = shard.astype(ml_dtypes.bfloat16)
        xlo = (shard - xhi.astype(np.float32)).astype(ml_dtypes.bfloat16)
        xhl = np.stack(
            [xhi, xlo, np.full(TOK_PER_CORE, 0.25, dtype=ml_dtypes.bfloat16)]
        )
        in_maps.append(
            {"xhl": np.ascontiguousarray(xhl), "pwcol": pwcol, "embhl": embhl}
        )
    return in_maps


def kernel(x, emb):
    global last_results
    from concourse.bass_utils import run_bass_kernel_spmd

    x = np.asarray(x)
    emb = np.asarray(emb, dtype=np.float32)
    orig_shape = x.shape
    x_flat = x.reshape(-1)
    assert x_flat.shape[0] == TOKENS
    x_f32 = x_flat.astype(np.float32)  # values < 8192, exact in f32
    in_maps = _make_in_maps(x_f32, emb)

    nc = _get_module()
    res = run_bass_kernel_spmd(nc, in_maps, core_ids=list(range(N_CORES)))
    last_results = res

    out = np.concatenate(
        [np.asarray(res.results[c]["out"]).astype(np.float32) for c in range(N_CORES)],
        axis=0,
    )
    return out.reshape(*orig_shape, D_MODEL)
